# revision 17
# baseline (speedup 1.0000x reference)
"""Bass/Tile TRN2 kernel for nn_Attention_3264175145281.

Computes, for each batch row b:
    energy[s] = encoder_outputs[b, s, :] @ W[0, :512]   (+ const(b), dropped)
    weights   = softmax(energy)
    context   = weights @ encoder_outputs[b]

The reference adds `hidden @ W[0, 512:] + bias` to every energy[s]; that term
is constant along s, and softmax is shift-invariant, so the output does not
depend on it.  We therefore stream encoder_outputs exactly once per core.

Sharding: batch dim across 8 NeuronCores (4 rows each), W replicated.
"""

import os
import sys

import numpy as np

for _p in ("/opt/trn_rl_repo", os.path.expanduser("~/.axon_site/_ro/trn_rl_repo")):
    if os.path.isdir(_p) and _p not in sys.path:
        sys.path.insert(0, _p)

from contextlib import ExitStack

import concourse.bacc as bacc
import concourse.bass as bass
import concourse.mybir as mybir
import concourse.tile as tile
from concourse.bass_utils import run_bass_kernel_spmd

B, S, ENC = 32, 4096, 512
NCORES = 8
B_LOC = B // NCORES          # 4 batch rows per core
P = 128                      # SBUF partitions
NCH = S // P                 # 32 chunks of 128 positions
GRP = 8                      # chunks per DMA group
NGRP = NCH // GRP            # 4 group DMAs (2 MiB each) per batch
F32 = mybir.dt.float32
F32R = mybir.dt.float32r     # 1 cyc/col on PE at N>=256 (vs 4 for fp32), ~14-bit mantissa


def build_program(n_b: int = B_LOC) -> bass.Bass:
    nc = bacc.Bacc("TRN2", target_bir_lowering=False, debug=False)

    x = nc.dram_tensor("x", [n_b, S, ENC], F32R, kind="ExternalInput").ap()
    wenc = nc.dram_tensor("wenc", [1, ENC], F32, kind="ExternalInput").ap()
    out = nc.dram_tensor("out", [n_b, ENC], F32, kind="ExternalOutput").ap()

    with tile.TileContext(nc) as tc, ExitStack() as ctx:
        const_pool = ctx.enter_context(tc.tile_pool(name="const", bufs=1))
        x_pool = ctx.enter_context(tc.tile_pool(name="xg", bufs=2 * NGRP))
        scr_pool = ctx.enter_context(tc.tile_pool(name="scr", bufs=4))
        stat_pool = ctx.enter_context(tc.tile_pool(name="stat", bufs=2))
        out_pool = ctx.enter_context(tc.tile_pool(name="outp", bufs=2))
        psum_pool = ctx.enter_context(tc.tile_pool(name="psum", bufs=2, space="PSUM"))

        # w_enc replicated to all 128 partitions (step-0 DMA broadcast).
        wb = const_pool.tile([P, ENC], F32, tag="wb")
        nc.sync.dma_start(wb[:], wenc[:, :].broadcast_to([P, ENC]))

        ones = const_pool.tile([P, 1], F32, tag="ones")
        nc.gpsimd.memset(ones[:], 1.0)

        for b in range(n_b):
            groups = []
            energy = stat_pool.tile([P, NCH], F32, tag="energy")
            for g in range(NGRP):
                # s = g*1024 + p*GRP + k  for partition p, chunk k: each
                # partition reads one contiguous 16 KiB run from DRAM.
                gx = x_pool.tile([P, GRP, ENC], F32R, tag="gx")
                src = x[b, g * P * GRP:(g + 1) * P * GRP, :]
                nc.sync.dma_start(gx[:], src.rearrange("(p k) e -> p k e", p=P))
                groups.append(gx)
                for k in range(GRP):
                    j = g * GRP + k
                    scr = scr_pool.tile([P, ENC], F32, tag="scr")
                    # energy[:, j] = sum_e x[:, e] * w_enc[e]  (one DVE pass)
                    nc.vector.scalar_tensor_tensor(
                        out=scr[:],
                        in0=gx[:, k, :].bitcast(F32),
                        scalar=1.0,
                        in1=wb[:],
                        op0=mybir.AluOpType.mult,
                        op1=mybir.AluOpType.mult,
                        accum_out=energy[:, j:j + 1],
                    )

            # p = exp(energy); rowsum[p] = sum_j p[p, j]
            p_t = stat_pool.tile([P, NCH], F32R, tag="p")
            rowsum = stat_pool.tile([P, 1], F32, tag="rowsum")
            nc.scalar.activation(
                p_t[:], energy[:], mybir.ActivationFunctionType.Exp,
                accum_out=rowsum[:],
            )

            # Z = sum over all positions (partition-sum of rowsum via PE).
            z_psum = psum_pool.tile([1, 1], F32, tag="z")
            nc.tensor.matmul(z_psum[:], rowsum[:], ones[:], start=True, stop=True)

            # context_raw = sum_j p[:, j]^T @ X_j  accumulated in PSUM.
            ctx_psum = psum_pool.tile([1, ENC], F32, tag="ctx")
            for g in range(NGRP):
                for k in range(GRP):
                    j = g * GRP + k
                    nc.tensor.matmul(
                        ctx_psum[:],
                        p_t[:, j:j + 1],
                        groups[g][:, k, :],
                        start=(j == 0),
                        stop=(j == NCH - 1),
                    )

            rz = stat_pool.tile([1, 1], F32, tag="rz")
            nc.vector.reciprocal(rz[:], z_psum[:])
            ot = out_pool.tile([1, ENC], F32, tag="ot")
            nc.vector.tensor_scalar_mul(ot[:], ctx_psum[:], rz[:])
            nc.sync.dma_start(out[b:b + 1, :], ot[:])

    nc.compile()
    return nc


_CACHED_NC = None


def _get_nc() -> bass.Bass:
    global _CACHED_NC
    if _CACHED_NC is None:
        _CACHED_NC = build_program()
    return _CACHED_NC


def run(inputs: dict, trace: bool = False, **kw):
    """Shard inputs, run on 8 cores, return (full_output, BassKernelResults)."""
    x_full = np.ascontiguousarray(np.asarray(inputs["encoder_outputs"], dtype=np.float32))
    w_full = np.ascontiguousarray(np.asarray(inputs["W"], dtype=np.float32))
    wenc = np.ascontiguousarray(w_full[:, :ENC])

    nc = _get_nc()
    in_maps = [
        {"x": np.ascontiguousarray(x_full[c * B_LOC:(c + 1) * B_LOC]), "wenc": wenc}
        for c in range(NCORES)
    ]
    res = run_bass_kernel_spmd(nc, in_maps, list(range(NCORES)), trace=trace, **kw)
    out = np.concatenate([res.results[c]["out"] for c in range(NCORES)], axis=0)
    return out.astype(np.float32), res


def kernel(encoder_outputs, hidden, W, b):
    out, _ = run({"encoder_outputs": encoder_outputs, "W": W})
    return out


# revision 19
# speedup vs baseline: 1.1989x; 1.1989x over previous
"""Bass/Tile TRN2 kernel for nn_Attention_3264175145281.

Computes, for each batch row b:
    energy[s] = encoder_outputs[b, s, :] @ W[0, :512]   (+ const(b), dropped)
    weights   = softmax(energy)
    context   = weights @ encoder_outputs[b]

The reference adds `hidden @ W[0, 512:] + bias` to every energy[s]; that term
is constant along s, and softmax is shift-invariant, so the output does not
depend on it.  We therefore stream encoder_outputs exactly once per core.

Sharding: batch dim across 8 NeuronCores (4 rows each), W replicated.
"""

import os
import sys

import numpy as np

for _p in ("/opt/trn_rl_repo", os.path.expanduser("~/.axon_site/_ro/trn_rl_repo")):
    if os.path.isdir(_p) and _p not in sys.path:
        sys.path.insert(0, _p)

from contextlib import ExitStack

import concourse.bacc as bacc
import concourse.bass as bass
import concourse.mybir as mybir
import concourse.tile as tile
from concourse.bass_utils import run_bass_kernel_spmd

B, S, ENC = 32, 4096, 512
NCORES = 8
B_LOC = B // NCORES          # 4 batch rows per core
P = 128                      # SBUF partitions
NCH = S // P                 # 32 chunks of 128 positions
GRP = 4                      # chunks per DMA group (1 MiB transfers)
NGRP = NCH // GRP            # 8 group DMAs per batch
EGRP = 8                     # chunks per exp/matmul wave
NEG = NCH // EGRP            # 4 waves per batch
F32 = mybir.dt.float32
F32R = mybir.dt.float32r     # 1 cyc/col on PE at N>=256 (vs 4 for fp32), ~14-bit mantissa


def build_program(n_b: int = B_LOC) -> bass.Bass:
    nc = bacc.Bacc("TRN2", target_bir_lowering=False, debug=False)

    x = nc.dram_tensor("x", [n_b, S, ENC], F32R, kind="ExternalInput").ap()
    wenc = nc.dram_tensor("wenc", [1, ENC], F32, kind="ExternalInput").ap()
    out = nc.dram_tensor("out", [n_b, ENC], F32, kind="ExternalOutput").ap()

    with tile.TileContext(nc) as tc, ExitStack() as ctx:
        const_pool = ctx.enter_context(tc.tile_pool(name="const", bufs=1))
        x_pool = ctx.enter_context(tc.tile_pool(name="xg", bufs=20))
        scr_pool = ctx.enter_context(tc.tile_pool(name="scr", bufs=4))
        stat_pool = ctx.enter_context(tc.tile_pool(name="stat", bufs=2))
        rs_pool = ctx.enter_context(tc.tile_pool(name="rs", bufs=2 * NEG))
        out_pool = ctx.enter_context(tc.tile_pool(name="outp", bufs=4))
        psum_pool = ctx.enter_context(tc.tile_pool(name="psum", bufs=3, space="PSUM"))

        # w_enc replicated to all 128 partitions (step-0 DMA broadcast).
        wb = const_pool.tile([P, ENC], F32, tag="wb")
        nc.sync.dma_start(wb[:], wenc[:, :].broadcast_to([P, ENC]))

        ones = const_pool.tile([P, 1], F32, tag="ones")
        nc.gpsimd.memset(ones[:], 1.0)

        for b in range(n_b):
            groups = []
            energy = stat_pool.tile([P, NCH], F32, tag="energy")
            p_t = stat_pool.tile([P, NCH], F32R, tag="p")
            ctx_psum = psum_pool.tile([1, ENC], F32, tag="ctx")
            z_psum = psum_pool.tile([1, 1], F32, tag="z")

            for g in range(NGRP):
                # s = g*P*GRP + p*GRP + k: each partition reads one
                # contiguous 8 KiB run from DRAM (1 MiB per dma_start).
                gx = x_pool.tile([P, GRP, ENC], F32R, tag="gx")
                src = x[b, g * P * GRP:(g + 1) * P * GRP, :]
                nc.sync.dma_start(gx[:], src.rearrange("(p k) e -> p k e", p=P))
                groups.append(gx)
                for k in range(GRP):
                    j = g * GRP + k
                    scr = scr_pool.tile([P, ENC], F32, tag="scr")
                    # energy[:, j] = sum_e x[:, e] * w_enc[e]  (one DVE pass)
                    nc.vector.scalar_tensor_tensor(
                        out=scr[:],
                        in0=gx[:, k, :].bitcast(F32),
                        scalar=1.0,
                        in1=wb[:],
                        op0=mybir.AluOpType.mult,
                        op1=mybir.AluOpType.mult,
                        accum_out=energy[:, j:j + 1],
                    )

                # After every EGRP chunks: exp wave + matmul wave, so the
                # PE work overlaps the next chunks' DMA/DVE instead of
                # serializing at the batch tail.
                if (g + 1) % (EGRP // GRP) == 0:
                    e = g // (EGRP // GRP)       # wave index 0..NEG-1
                    j0 = e * EGRP
                    rowsum = rs_pool.tile([P, 1], F32, tag="rowsum")
                    nc.scalar.activation(
                        p_t[:, j0:j0 + EGRP], energy[:, j0:j0 + EGRP],
                        mybir.ActivationFunctionType.Exp,
                        accum_out=rowsum[:],
                    )
                    nc.tensor.matmul(
                        z_psum[:], rowsum[:], ones[:],
                        start=(e == 0), stop=(e == NEG - 1),
                    )
                    for j in range(j0, j0 + EGRP):
                        nc.tensor.matmul(
                            ctx_psum[:],
                            p_t[:, j:j + 1],
                            groups[j // GRP][:, j % GRP, :],
                            start=(j == 0),
                            stop=(j == NCH - 1),
                        )

            rz = stat_pool.tile([1, 1], F32, tag="rz")
            nc.vector.reciprocal(rz[:], z_psum[:])
            ot = out_pool.tile([1, ENC], F32, tag="ot")
            # final scale on the (idle) scalar engine: out = ctx * (1/Z)
            nc.scalar.activation(
                ot[:], ctx_psum[:], mybir.ActivationFunctionType.Copy,
                scale=rz[:],
            )
            nc.sync.dma_start(out[b:b + 1, :], ot[:])

    nc.compile()
    return nc


_CACHED_NC = None


def _get_nc() -> bass.Bass:
    global _CACHED_NC
    if _CACHED_NC is None:
        _CACHED_NC = build_program()
    return _CACHED_NC


def run(inputs: dict, trace: bool = False, **kw):
    """Shard inputs, run on 8 cores, return (full_output, BassKernelResults)."""
    x_full = np.ascontiguousarray(np.asarray(inputs["encoder_outputs"], dtype=np.float32))
    w_full = np.ascontiguousarray(np.asarray(inputs["W"], dtype=np.float32))
    wenc = np.ascontiguousarray(w_full[:, :ENC])

    nc = _get_nc()
    in_maps = [
        {"x": np.ascontiguousarray(x_full[c * B_LOC:(c + 1) * B_LOC]), "wenc": wenc}
        for c in range(NCORES)
    ]
    res = run_bass_kernel_spmd(nc, in_maps, list(range(NCORES)), trace=trace, **kw)
    out = np.concatenate([res.results[c]["out"] for c in range(NCORES)], axis=0)
    return out.astype(np.float32), res


def kernel(encoder_outputs, hidden, W, b):
    out, _ = run({"encoder_outputs": encoder_outputs, "W": W})
    return out
